# revision 4
# baseline (speedup 1.0000x reference)
"""Locally banded sparse attention (window=64) on 8 Trainium2 NeuronCores.

Sequence-parallel: each core owns 256 contiguous query positions and
receives a 384-row x chunk (its 256 rows + 64-row halo on each side,
zero-padded at the sequence edges) plus a full replica of the four
projection matrices.  No device collectives are needed.

Per-core device kernel (all fp32):
  1. qT/kT (head-transposed, d on partitions) and v (natural, keys on
     partitions) projections via PE matmuls.
  2. For each of 2 query tiles x 8 heads: a dense 128x256 score block
     (the 129-wide band of a 128-query tile spans exactly 256 contiguous
     keys), additive band/validity mask, softmax along the free axis,
     PE transpose of the probabilities, and P@V accumulation.
  3. Output projection producing outT [512, 256]; the host transposes
     and concatenates the 8 chunks.
"""

import numpy as np

import concourse.bass as bass
import concourse.tile as tile
from concourse import bacc, mybir
from concourse import bass_utils
from concourse.bass import ts, ds
from concourse.masks import make_identity

F32 = mybir.dt.float32
N_CORES = 8
S = 2048
D = 512
H = 8
DK = 64
W = 64
SCALE = 1.0 / np.sqrt(DK)
SEQ_PER_CORE = S // N_CORES          # 256
CHUNK = SEQ_PER_CORE + 2 * W         # 384 rows of k/v context per core
NEG = -1.0e30

_CACHE = {}


def _build_program():
    nc = bacc.Bacc("TRN2", target_bir_lowering=False, debug=False,
                   num_devices=N_CORES)

    xT = nc.dram_tensor("xT", [D, CHUNK], F32, kind="ExternalInput").ap()
    wqT = nc.dram_tensor("wqT", [D, D], F32, kind="ExternalInput").ap()
    wkT = nc.dram_tensor("wkT", [D, D], F32, kind="ExternalInput").ap()
    wvT = nc.dram_tensor("wvT", [D, D], F32, kind="ExternalInput").ap()
    woT = nc.dram_tensor("woT", [D, D], F32, kind="ExternalInput").ap()
    mask = nc.dram_tensor("mask", [2, 128, 256], F32, kind="ExternalInput").ap()
    # biases, pre-reshaped to [128, 4] (bias[g*128+p] -> [p, g]); bq pre-scaled
    bq = nc.dram_tensor("bq", [128, 4], F32, kind="ExternalInput").ap()
    bk = nc.dram_tensor("bk", [128, 4], F32, kind="ExternalInput").ap()
    bv = nc.dram_tensor("bv", [128, 4], F32, kind="ExternalInput").ap()
    bo = nc.dram_tensor("bo", [128, 4], F32, kind="ExternalInput").ap()
    outT = nc.dram_tensor("outT", [D, SEQ_PER_CORE], F32,
                          kind="ExternalOutput").ap()

    QLO, QHI = W, W + SEQ_PER_CORE   # query rows inside the chunk

    with tile.TileContext(nc) as tc:
        with (
            tc.tile_pool(name="const", bufs=1) as cpool,
            tc.tile_pool(name="proj_ps", bufs=2, space="PSUM") as proj_ps,
            tc.tile_pool(name="s_ps", bufs=2, space="PSUM") as s_ps,
            tc.tile_pool(name="pt_ps", bufs=2, space="PSUM") as pt_ps,
            tc.tile_pool(name="av_ps", bufs=2, space="PSUM") as av_ps,
            tc.tile_pool(name="soft", bufs=3) as soft,
            tc.tile_pool(name="small", bufs=4) as small,
        ):
            # ---- persistent SBUF tiles -------------------------------
            def persist(shape, tag):
                return cpool.tile(shape, F32, tag=tag, name=tag)

            x_sb = [persist([128, CHUNK], f"x{k}") for k in range(4)]
            wq_sb = [persist([128, D], f"wq{k}") for k in range(4)]
            wk_sb = [persist([128, D], f"wk{k}") for k in range(4)]
            wv_sb = [persist([128, D], f"wv{k}") for k in range(4)]
            wo_sb = [persist([128, D], f"wo{k}") for k in range(4)]
            m_sb = [persist([128, 256], f"m{t}") for t in range(2)]
            q_sb = [persist([128, SEQ_PER_CORE], f"q{g}") for g in range(4)]
            k_sb = [persist([128, CHUNK], f"k{g}") for g in range(4)]
            v_sb = [persist([128, D], f"v{r}") for r in range(3)]
            a_sb = [persist([128, SEQ_PER_CORE], f"a{g}") for g in range(4)]
            o_sb = [persist([128, SEQ_PER_CORE], f"o{g}") for g in range(4)]
            bq_sb = persist([128, 4], "bq")
            bk_sb = persist([128, 4], "bk")
            bv_sb = persist([128, 4], "bv")
            bo_sb = persist([128, 4], "bo")
            ident = persist([128, 128], "ident")

            make_identity(nc, ident[:])

            for k in range(4):
                nc.sync.dma_start(x_sb[k][:], xT[ts(k, 128), :])
                nc.sync.dma_start(wk_sb[k][:], wkT[ts(k, 128), :])
                nc.sync.dma_start(wq_sb[k][:], wqT[ts(k, 128), :])
                nc.sync.dma_start(wv_sb[k][:], wvT[ts(k, 128), :])
                nc.sync.dma_start(wo_sb[k][:], woT[ts(k, 128), :])
            for t in range(2):
                nc.sync.dma_start(m_sb[t][:], mask[t, :, :])
            nc.sync.dma_start(bq_sb[:], bq[:, :])
            nc.sync.dma_start(bk_sb[:], bk[:, :])
            nc.sync.dma_start(bv_sb[:], bv[:, :])
            nc.sync.dma_start(bo_sb[:], bo[:, :])

            # ---- projections ----------------------------------------
            # kT[o, j] / qT[o, r]: accumulate over d-chunks kk
            for g in range(4):
                ps = proj_ps.tile([128, 512], F32, tag="proj", name="proj")
                for kk in range(4):
                    nc.tensor.matmul(ps[:, :CHUNK], wk_sb[kk][:, ts(g, 128)],
                                     x_sb[kk][:], start=(kk == 0),
                                     stop=(kk == 3))
                nc.scalar.activation(k_sb[g][:], ps[:, :CHUNK],
                                     mybir.ActivationFunctionType.Identity,
                                     bias=bk_sb[:, g:g + 1])
            for g in range(4):
                ps = proj_ps.tile([128, 512], F32, tag="proj", name="proj")
                for kk in range(4):
                    nc.tensor.matmul(ps[:, :SEQ_PER_CORE],
                                     wq_sb[kk][:, ts(g, 128)],
                                     x_sb[kk][:, QLO:QHI], start=(kk == 0),
                                     stop=(kk == 3))
                # q is pre-scaled by 1/sqrt(dk); bq arrives pre-scaled too
                nc.scalar.activation(q_sb[g][:], ps[:, :SEQ_PER_CORE],
                                     mybir.ActivationFunctionType.Identity,
                                     bias=bq_sb[:, g:g + 1], scale=SCALE)
            # v natural ([keys, d]): lhsT = x chunk cols, rhs = wvT
            for r in range(3):
                ps = proj_ps.tile([128, 512], F32, tag="proj", name="proj")
                for kk in range(4):
                    nc.tensor.matmul(ps[:], x_sb[kk][:, ts(r, 128)],
                                     wv_sb[kk][:],
                                     start=(kk == 0), stop=(kk == 3))
                nc.scalar.activation(v_sb[r][:], ps[:],
                                     mybir.ActivationFunctionType.Identity)

            # ---- banded attention -----------------------------------
            for t in range(2):
                for h in range(8):
                    g, po = h // 2, (h % 2) * 64
                    sps = s_ps.tile([128, 256], F32, tag="s", name="s")
                    nc.tensor.matmul(sps[:],
                                     q_sb[g][ds(po, 64), ts(t, 128)],
                                     k_sb[g][ds(po, 64), ds(t * 128, 256)],
                                     start=True, stop=True)
                    p = soft.tile([128, 256], F32, tag="p", name="p")
                    nc.vector.tensor_add(p[:], sps[:], m_sb[t][:])
                    nmax = small.tile([128, 1], F32, tag="nmax", name="nmax")
                    nc.vector.reduce_max(out=nmax[:], in_=p[:],
                                         axis=mybir.AxisListType.X,
                                         negate=True)
                    rsum = small.tile([128, 1], F32, tag="rsum", name="rsum")
                    nc.scalar.activation(p[:], p[:],
                                         mybir.ActivationFunctionType.Exp,
                                         bias=nmax[:], accum_out=rsum[:])
                    rcp = small.tile([128, 1], F32, tag="rcp", name="rcp")
                    nc.vector.reciprocal(rcp[:], rsum[:])
                    nc.vector.tensor_scalar_mul(p[:], p[:], rcp[:])
                    av = av_ps.tile([64, 128], F32, tag="av", name="av")
                    for jb in range(2):
                        ptp = pt_ps.tile([128, 128], F32, tag="pt", name="pt")
                        nc.tensor.transpose(ptp[:], p[:, ts(jb, 128)],
                                            ident[:])
                        pts = soft.tile([128, 128], F32, tag="pts", name="pts")
                        nc.vector.tensor_copy(pts[:], ptp[:])
                        nc.tensor.matmul(av[:],
                                         v_sb[t + jb][:, ds(h * DK, DK)],
                                         pts[:], start=(jb == 0),
                                         stop=(jb == 1))
                    nc.scalar.activation(a_sb[g][ds(po, 64), ts(t, 128)],
                                         av[:],
                                         mybir.ActivationFunctionType.Identity,
                                         bias=bv_sb[ds(po, 64), g:g + 1])

            # ---- output projection ----------------------------------
            for g in range(4):
                ps = proj_ps.tile([128, 512], F32, tag="proj", name="proj")
                for kk in range(4):
                    nc.tensor.matmul(ps[:, :SEQ_PER_CORE],
                                     wo_sb[kk][:, ts(g, 128)],
                                     a_sb[kk][:], start=(kk == 0),
                                     stop=(kk == 3))
                nc.scalar.activation(o_sb[g][:], ps[:, :SEQ_PER_CORE],
                                     mybir.ActivationFunctionType.Identity,
                                     bias=bo_sb[:, g:g + 1])
                nc.sync.dma_start(outT[ts(g, 128), :], o_sb[g][:])

    nc.compile()
    return nc


def _band_mask(tile_idx):
    """Additive mask [128, 256] for global query tile `tile_idx` (0..15)."""
    r = np.arange(128)[:, None]
    j = np.arange(256)[None, :]
    ok = (j >= r) & (j <= r + 2 * W)
    key_global = tile_idx * 128 - W + j
    ok &= (key_global >= 0) & (key_global < S)
    return np.where(ok, 0.0, NEG).astype(np.float32)


def _prep_inputs(x, Wq, bq, Wk, bk, Wv, bv, Wo, bo):
    wqT = np.ascontiguousarray(Wq.T.astype(np.float32))
    wkT = np.ascontiguousarray(Wk.T.astype(np.float32))
    wvT = np.ascontiguousarray(Wv.T.astype(np.float32))
    woT = np.ascontiguousarray(Wo.T.astype(np.float32))

    def resh(b, scale=1.0):
        return np.ascontiguousarray(
            (np.asarray(b, np.float32) * scale).reshape(4, 128).T)

    bq_r, bk_r = resh(bq, SCALE), resh(bk)
    bv_r, bo_r = resh(bv), resh(bo)

    in_maps = []
    for c in range(N_CORES):
        klo = c * SEQ_PER_CORE - W
        lo, hi = max(0, klo), min(S, klo + CHUNK)
        xT_c = np.zeros((D, CHUNK), np.float32)
        xT_c[:, lo - klo:hi - klo] = x[0, lo:hi, :].T
        m = np.stack([_band_mask(c * 2), _band_mask(c * 2 + 1)])
        in_maps.append({
            "xT": np.ascontiguousarray(xT_c), "mask": np.ascontiguousarray(m),
            "wqT": wqT, "wkT": wkT, "wvT": wvT, "woT": woT,
            "bq": bq_r, "bk": bk_r, "bv": bv_r, "bo": bo_r,
        })
    return in_maps


def kernel(x, Wq, bq, Wk, bk, Wv, bv, Wo, bo):
    if "nc" not in _CACHE:
        _CACHE["nc"] = _build_program()
    nc = _CACHE["nc"]
    in_maps = _prep_inputs(x, Wq, bq, Wk, bk, Wv, bv, Wo, bo)
    res = bass_utils.run_bass_kernel_spmd(nc, in_maps,
                                          core_ids=list(range(N_CORES)))
    out = np.empty((1, S, D), np.float32)
    for c in range(N_CORES):
        out[0, c * SEQ_PER_CORE:(c + 1) * SEQ_PER_CORE, :] = \
            res.results[c]["outT"].T
    return out
